# revision 25
# baseline (speedup 1.0000x reference)
"""Trainium2 Bass kernel for a 16-head MHA block (B=1, S=4096, H=1024).

Sharding: tensor-parallel over heads — each of the 8 cores owns 2 heads
(128 of the 1024 Wq/Wk/Wv output channels) and computes 512 rows of the
final (scrambled) output; the host concatenates the row blocks.

Per-core dataflow (all matmuls in bf16, fp32 PSUM accumulation):
  qT/kT = relu(W @ x.T + b)      layout [128 chan, 4096 seq]   (transposed)
  v     = relu(x @ W.T + b)      layout [4096 seq, 128 chan]   (natural)
  S_T[t,s]  = sum_d kT[d,t] qT[d,s]          (2 heads row-packed, K=64)
  E = exp(S_T/8 + shift)                      (ScalarE, PSUM->SBUF bf16)
  num/den   = sum_t [v|1][t,d'] E[t,s]       (M=65 matmul: row 64 = denom)
  a_T = num * recip(den)                      (normalize at the end, DVE)
  out rows (256n + 4d + j, m) = a_T[d, 1024j+m] + queries[...]
"""

import numpy as np
import ml_dtypes

import concourse.bass as bass
import concourse.tile as tile
from concourse import bacc, mybir
from concourse.bass import ds, ts
from concourse.bass_utils import run_bass_kernel_spmd

BF16 = ml_dtypes.bfloat16
S = 4096
H = 1024
NCORES = 8
OC = H // NCORES  # 128 output channels (2 heads) per core
SBLK = 512  # s-block width
NSB = S // SBLK  # 8
NT = S // 128  # 32 t-chunks
NKC = H // 128  # 8 contraction chunks for projections
EXP_SHIFT = -4.0  # exp(s/8 + shift): softmax-invariant guard shift

_CACHE = {}


def _build_nc():
    f32 = mybir.dt.float32
    bf16 = mybir.dt.bfloat16
    add = mybir.AluOpType.add
    mult = mybir.AluOpType.mult
    amax = mybir.AluOpType.max
    Exp = mybir.ActivationFunctionType.Exp
    Ln = mybir.ActivationFunctionType.Ln

    nc = bacc.Bacc("TRN2", target_bir_lowering=False, debug=False)

    xq = nc.dram_tensor("xq", [H, S], bf16, kind="ExternalInput").ap()
    xk = nc.dram_tensor("xk", [H, S], bf16, kind="ExternalInput").ap()
    xv = nc.dram_tensor("xv", [H, S], bf16, kind="ExternalInput").ap()
    wq = nc.dram_tensor("wq", [H, OC], bf16, kind="ExternalInput").ap()
    wk = nc.dram_tensor("wk", [H, OC], bf16, kind="ExternalInput").ap()
    wv = nc.dram_tensor("wv", [H, OC], bf16, kind="ExternalInput").ap()
    bq = nc.dram_tensor("bq", [OC, 1], f32, kind="ExternalInput").ap()
    bk = nc.dram_tensor("bk", [OC, 1], f32, kind="ExternalInput").ap()
    bv = nc.dram_tensor("bv", [OC, 1], f32, kind="ExternalInput").ap()
    qres = nc.dram_tensor("qres", [512, H], f32, kind="ExternalInput").ap()
    out = nc.dram_tensor("out", [512, H], f32, kind="ExternalOutput").ap()

    # [H, S] -> [128, chunk, S]; [H, OC] -> [128, chunk, OC]
    xq_r = xq.rearrange("(c p) s -> p c s", p=128)
    xk_r = xk.rearrange("(c p) s -> p c s", p=128)
    xv_r = xv.rearrange("(c p) s -> p c s", p=128)
    wq_r = wq.rearrange("(c p) o -> p c o", p=128)
    wk_r = wk.rearrange("(c p) o -> p c o", p=128)
    wv_r = wv.rearrange("(c p) o -> p c o", p=128)
    # residual/output rows: local row = 256*hl + 4*d + j
    qres_r = qres.rearrange("(hl d j) m -> hl d j m", hl=2, d=64)
    out_r = out.rearrange("(hl d j) m -> hl d j m", hl=2, d=64)

    with tile.TileContext(nc) as tc:
        with (
            tc.tile_pool(name="const", bufs=1) as constp,
            tc.tile_pool(name="persist", bufs=1) as persist,
            tc.tile_pool(name="stage", bufs=2) as stage,
            tc.tile_pool(name="exps", bufs=6) as expp,
            tc.tile_pool(name="epi", bufs=2) as epi,
            tc.tile_pool(name="ps_a", bufs=2, space="PSUM") as ps_a,
            tc.tile_pool(name="ps_av", bufs=2, space="PSUM") as ps_av,
        ):
            # ---- constants ----
            wq_sb = constp.tile([128, NKC, OC], bf16)
            wk_sb = constp.tile([128, NKC, OC], bf16)
            wv_sb = constp.tile([128, NKC, OC], bf16)
            nc.sync.dma_start(wq_sb[:], wq_r)
            nc.sync.dma_start(wk_sb[:], wk_r)
            nc.sync.dma_start(wv_sb[:], wv_r)
            bq_sb = constp.tile([OC, 1], f32)
            bk_sb = constp.tile([OC, 1], f32)
            bv_sb = constp.tile([OC, 1], f32)
            nc.sync.dma_start(bq_sb[:], bq)
            nc.sync.dma_start(bk_sb[:], bk)
            nc.sync.dma_start(bv_sb[:], bv)
            shift_sb = constp.tile([128, 1], f32)
            nc.vector.memset(shift_sb[:], EXP_SHIFT)

            qT_sb = persist.tile([128, S], bf16)
            kT_sb = persist.tile([128, S], bf16)
            # per (ti, head): 128 cols = [v[t, d0..d63] | ones | pad63]
            # (block stride 256B: dma_start_transpose needs aligned dst)
            v_sb = persist.tile([128, NT * 2 * 128], bf16)
            v_sb_r = v_sb.rearrange("p (t h w) -> p t h w", t=NT, h=2)
            nc.vector.memset(v_sb_r[:, :, :, 64:65], 1.0)

            # ---- k/v projections (per s-block) ----
            for sb in range(NSB):
                ss = ds(sb * SBLK, SBLK)
                xk_st = stage.tile([128, NKC, SBLK], bf16, name="xk_st")
                xv_st = stage.tile([128, NKC, SBLK], bf16, name="xv_st")
                nc.sync.dma_start(xk_st[:], xk_r[:, :, ss])
                nc.sync.dma_start(xv_st[:], xv_r[:, :, ss])

                kp = ps_a.tile([128, 1024], f32, tag="ps_big", name="kp")
                for ci in range(NKC):
                    nc.tensor.matmul(
                        kp[:, :SBLK], wk_sb[:, ci, :], xk_st[:, ci, :],
                        start=(ci == 0), stop=(ci == NKC - 1),
                    )
                nc.vector.tensor_scalar(
                    kT_sb[:, ss], kp[:, :SBLK], bk_sb[:], 0.0, add, amax
                )
                vp = ps_a.tile([128, 1024], f32, tag="ps_big", name="vp")
                for ci in range(NKC):
                    nc.tensor.matmul(
                        vp[:, :SBLK], wv_sb[:, ci, :], xv_st[:, ci, :],
                        start=(ci == 0), stop=(ci == NKC - 1),
                    )
                vT_st = stage.tile([128, SBLK], bf16, name="vT_st")
                nc.vector.tensor_scalar(
                    vT_st[:], vp[:, :SBLK], bv_sb[:], 0.0, add, amax
                )
                for tj in range(4):
                    ti = sb * 4 + tj
                    for hl in range(2):
                        nc.sync.dma_start_transpose(
                            v_sb_r[:, ti, hl, 0:64],
                            vT_st[ds(hl * 64, 64), ts(tj, 128)],
                        )

            # ---- attention (per s-block) ----
            def q_proj(sb):
                ss = ds(sb * SBLK, SBLK)
                xq_st = stage.tile([128, NKC, SBLK], bf16, name="xq_st")
                nc.sync.dma_start(xq_st[:], xq_r[:, :, ss])
                qp = ps_a.tile([128, 1024], f32, tag="ps_big", name="qp")
                for ci in range(NKC):
                    nc.tensor.matmul(
                        qp[:, :SBLK], wq_sb[:, ci, :], xq_st[:, ci, :],
                        start=(ci == 0), stop=(ci == NKC - 1),
                    )
                nc.vector.tensor_scalar(
                    qT_sb[:, ss], qp[:, :SBLK], bq_sb[:], 0.0, add, amax
                )

            q_proj(0)
            for sb in range(NSB):
                ss = ds(sb * SBLK, SBLK)
                if sb + 1 < NSB:
                    q_proj(sb + 1)
                # [0:65, 0:512] = head0 num+den, [0:65, 512:1024] = head1
                av = ps_av.tile([128, 1024], f32, name="av")
                for ti in range(NT):
                    tt = ds(ti * 128, 128)
                    sc = ps_a.tile([128, 1024], f32, tag="ps_big", name="sc")
                    for hl in range(2):
                        hh = ds(hl * 64, 64)
                        nc.tensor.matmul(
                            sc[:, ts(hl, SBLK)], kT_sb[hh, tt], qT_sb[hh, ss],
                            start=True, stop=True,
                        )
                    ex = expp.tile([128, 1024], bf16, name="ex")
                    nc.scalar.activation(
                        ex[:], sc[:], Exp, bias=shift_sb[:], scale=0.125
                    )
                    for hl in range(2):
                        nc.tensor.matmul(
                            av[0:65, ts(hl, SBLK)],
                            v_sb_r[:, ti, hl, 0:65],
                            ex[:, ts(hl, SBLK)],
                            start=(ti == 0), stop=(ti == NT - 1),
                        )

                # ---- epilogue: normalize + residual + store ----
                j = sb // 2
                mm = ds((sb % 2) * SBLK, SBLK)
                den64 = epi.tile([65, 1024], f32, name="den64")
                nc.vector.tensor_copy(den64[64:65, :], av[64:65, :])
                den0 = epi.tile([1, 1024], f32, name="den0")
                nc.sync.dma_start(den0[:], den64[64:65, :])
                bcd = epi.tile([64, 1024], f32, name="bcd")
                nc.gpsimd.partition_broadcast(bcd[:], den0[:])
                bcs = epi.tile([64, 1024], f32, name="bcs")
                nc.vector.reciprocal_approx_fast(bcs[:], bcd[:])
                qrt = epi.tile([64, 1024], f32, name="qrt")
                for hl in range(2):
                    nc.sync.dma_start(
                        qrt[:, ts(hl, SBLK)], qres_r[hl, :, j, mm]
                    )
                prod = epi.tile([64, 1024], f32, name="prod")
                nc.vector.tensor_tensor(prod[:], av[0:64, :], bcs[:], mult)
                outt = epi.tile([64, 1024], f32, name="outt")
                nc.vector.tensor_tensor(outt[:], prod[:], qrt[:], add)
                for hl in range(2):
                    nc.sync.dma_start(
                        out_r[hl, :, j, mm], outt[:, ts(hl, SBLK)]
                    )

    nc.compile()
    return nc


def _get_nc():
    if "nc" not in _CACHE:
        _CACHE["nc"] = _build_nc()
    return _CACHE["nc"]


def kernel(queries, keys, values, Wq_w, Wq_b, Wk_w, Wk_b, Wv_w, Wv_b, **kw):
    nc = _get_nc()
    q2 = np.asarray(queries, np.float32).reshape(S, H)
    k2 = np.asarray(keys, np.float32).reshape(S, H)
    v2 = np.asarray(values, np.float32).reshape(S, H)
    xqT = np.ascontiguousarray(q2.T).astype(BF16)
    xkT = np.ascontiguousarray(k2.T).astype(BF16)
    xvT = np.ascontiguousarray(v2.T).astype(BF16)

    in_maps = []
    for c in range(NCORES):
        o = slice(OC * c, OC * (c + 1))
        in_maps.append(
            {
                "xq": xqT,
                "xk": xkT,
                "xv": xvT,
                "wq": np.ascontiguousarray(np.asarray(Wq_w)[o].T).astype(BF16),
                "wk": np.ascontiguousarray(np.asarray(Wk_w)[o].T).astype(BF16),
                "wv": np.ascontiguousarray(np.asarray(Wv_w)[o].T).astype(BF16),
                "bq": np.asarray(Wq_b, np.float32)[o].reshape(OC, 1),
                "bk": np.asarray(Wk_b, np.float32)[o].reshape(OC, 1),
                "bv": np.asarray(Wv_b, np.float32)[o].reshape(OC, 1),
                "qres": np.ascontiguousarray(q2[512 * c : 512 * (c + 1)]),
            }
        )

    res = run_bass_kernel_spmd(
        nc, in_maps, list(range(NCORES)), **_CACHE.get("run_kwargs", {})
    )
    _CACHE["last_results"] = res
    full = np.concatenate([res.results[c]["out"] for c in range(NCORES)], axis=0)
    return full.reshape(1, S, H)


# revision 26
# speedup vs baseline: 1.2359x; 1.2359x over previous
"""Trainium2 Bass kernel for a 16-head MHA block (B=1, S=4096, H=1024).

Sharding: tensor-parallel over heads — each of the 8 cores owns 2 heads
(128 of the 1024 Wq/Wk/Wv output channels) and computes 512 rows of the
final (scrambled) output; the host concatenates the row blocks.

Per-core dataflow (all matmuls in bf16, fp32 PSUM accumulation):
  qT/kT = relu(W @ x.T + b)      layout [128 chan, 4096 seq]   (transposed)
  v     = relu(x @ W.T + b)      layout [4096 seq, 128 chan]   (natural)
  S_T[t,s]  = sum_d kT[d,t] qT[d,s]          (2 heads row-packed, K=64)
  E = exp(S_T/8 + shift)                      (ScalarE, PSUM->SBUF bf16)
  num/den   = sum_t [v|1][t,d'] E[t,s]       (M=65 matmul: row 64 = denom)
  a_T = num * recip(den)                      (normalize at the end, DVE)
  out rows (256n + 4d + j, m) = a_T[d, 1024j+m] + queries[...]
"""

import numpy as np
import ml_dtypes

import concourse.bass as bass
import concourse.tile as tile
from concourse import bacc, mybir
from concourse.bass import ds, ts
from concourse.bass_utils import run_bass_kernel_spmd

BF16 = ml_dtypes.bfloat16
S = 4096
H = 1024
NCORES = 8
OC = H // NCORES  # 128 output channels (2 heads) per core
SBLK = 512  # s-block width
NSB = S // SBLK  # 8
NT = S // 128  # 32 t-chunks
NKC = H // 128  # 8 contraction chunks for projections
EXP_SHIFT = -4.0  # exp(s/8 + shift): softmax-invariant guard shift

_CACHE = {}


def _build_nc():
    f32 = mybir.dt.float32
    bf16 = mybir.dt.bfloat16
    add = mybir.AluOpType.add
    mult = mybir.AluOpType.mult
    amax = mybir.AluOpType.max
    Exp = mybir.ActivationFunctionType.Exp
    Ln = mybir.ActivationFunctionType.Ln

    nc = bacc.Bacc("TRN2", target_bir_lowering=False, debug=False)

    xq = nc.dram_tensor("xq", [H, S], bf16, kind="ExternalInput").ap()
    xk = nc.dram_tensor("xk", [H, S], bf16, kind="ExternalInput").ap()
    xv = nc.dram_tensor("xv", [H, S], bf16, kind="ExternalInput").ap()
    wq = nc.dram_tensor("wq", [H, OC], bf16, kind="ExternalInput").ap()
    wk = nc.dram_tensor("wk", [H, OC], bf16, kind="ExternalInput").ap()
    wv = nc.dram_tensor("wv", [H, OC], bf16, kind="ExternalInput").ap()
    bq = nc.dram_tensor("bq", [OC, 1], f32, kind="ExternalInput").ap()
    bk = nc.dram_tensor("bk", [OC, 1], f32, kind="ExternalInput").ap()
    bv = nc.dram_tensor("bv", [1, OC], bf16, kind="ExternalInput").ap()
    qres = nc.dram_tensor("qres", [512, H], f32, kind="ExternalInput").ap()
    out = nc.dram_tensor("out", [512, H], f32, kind="ExternalOutput").ap()

    # [H, S] -> [128, chunk, S]; [H, OC] -> [128, chunk, OC]
    xq_r = xq.rearrange("(c p) s -> p c s", p=128)
    xk_r = xk.rearrange("(c p) s -> p c s", p=128)
    xv_r = xv.rearrange("(c p) s -> p c s", p=128)
    wq_r = wq.rearrange("(c p) o -> p c o", p=128)
    wk_r = wk.rearrange("(c p) o -> p c o", p=128)
    wv_r = wv.rearrange("(c p) o -> p c o", p=128)
    # residual/output rows: local row = 256*hl + 4*d + j
    qres_r = qres.rearrange("(hl d j) m -> hl d j m", hl=2, d=64)
    out_r = out.rearrange("(hl d j) m -> hl d j m", hl=2, d=64)

    with tile.TileContext(nc) as tc:
        with (
            tc.tile_pool(name="const", bufs=1) as constp,
            tc.tile_pool(name="persist", bufs=1) as persist,
            tc.tile_pool(name="stage", bufs=2) as stage,
            tc.tile_pool(name="exps", bufs=6) as expp,
            tc.tile_pool(name="epi", bufs=2) as epi,
            tc.tile_pool(name="ps_a", bufs=2, space="PSUM") as ps_a,
            tc.tile_pool(name="ps_av", bufs=2, space="PSUM") as ps_av,
        ):
            # ---- constants ----
            wq_sb = constp.tile([128, NKC, OC], bf16)
            wk_sb = constp.tile([128, NKC, OC], bf16)
            wv_sb = constp.tile([128, NKC, OC], bf16)
            nc.sync.dma_start(wq_sb[:], wq_r)
            nc.sync.dma_start(wk_sb[:], wk_r)
            nc.sync.dma_start(wv_sb[:], wv_r)
            bq_sb = constp.tile([OC, 1], f32)
            bk_sb = constp.tile([OC, 1], f32)
            bv_sb = constp.tile([1, OC], bf16)
            nc.sync.dma_start(bq_sb[:], bq)
            nc.sync.dma_start(bk_sb[:], bk)
            nc.sync.dma_start(bv_sb[:], bv)
            ones_rowb = constp.tile([1, 128], bf16)
            nc.vector.memset(ones_rowb[:], 1.0)
            shift_sb = constp.tile([128, 1], f32)
            nc.vector.memset(shift_sb[:], EXP_SHIFT)

            qT_sb = persist.tile([128, S], bf16)
            kT_sb = persist.tile([128, S], bf16)
            # per (ti, head): 128 cols = [v[t, d0..d63] | ones | pad63]
            # (block stride 256B: dma_start_transpose needs aligned dst)
            v_sb = persist.tile([128, NT * 2 * 128], bf16)
            v_sb_r = v_sb.rearrange("p (t h w) -> p t h w", t=NT, h=2)
            nc.vector.memset(v_sb_r[:, :, :, 64:65], 1.0)

            # ---- k/v projections (per s-block) ----
            for sb in range(NSB):
                ss = ds(sb * SBLK, SBLK)
                xk_st = stage.tile([128, NKC, SBLK], bf16, name="xk_st")
                xv_st = stage.tile([128, NKC, SBLK], bf16, name="xv_st")
                nc.sync.dma_start(xk_st[:], xk_r[:, :, ss])
                nc.sync.dma_start(xv_st[:], xv_r[:, :, ss])

                kp = ps_a.tile([128, 1024], f32, tag="ps_big", name="kp")
                for ci in range(NKC):
                    nc.tensor.matmul(
                        kp[:, :SBLK], wk_sb[:, ci, :], xk_st[:, ci, :],
                        start=(ci == 0), stop=(ci == NKC - 1),
                    )
                nc.vector.tensor_scalar(
                    kT_sb[:, ss], kp[:, :SBLK], bk_sb[:], 0.0, add, amax
                )
                for tj in range(4):
                    ti = sb * 4 + tj
                    vp = ps_a.tile([128, 1024], f32, tag="ps_big", name="vp")
                    for ci in range(NKC):
                        nc.tensor.matmul(
                            vp[:, 0:128], xv_st[:, ci, ts(tj, 128)], wv_sb[:, ci, :],
                            start=(ci == 0), stop=False,
                        )
                    nc.tensor.matmul(
                        vp[:, 0:128], ones_rowb[:1, :], bv_sb[:1, :],
                        start=False, stop=True,
                    )
                    nc.vector.tensor_scalar_max(
                        v_sb_r[:, ti, :, 0:64],
                        vp[:, 0:128].rearrange("p (h w) -> p h w", h=2),
                        0.0,
                    )

            # ---- attention (per s-block) ----
            def q_proj(sb):
                ss = ds(sb * SBLK, SBLK)
                xq_st = stage.tile([128, NKC, SBLK], bf16, name="xq_st")
                nc.sync.dma_start(xq_st[:], xq_r[:, :, ss])
                qp = ps_a.tile([128, 1024], f32, tag="ps_big", name="qp")
                for ci in range(NKC):
                    nc.tensor.matmul(
                        qp[:, :SBLK], wq_sb[:, ci, :], xq_st[:, ci, :],
                        start=(ci == 0), stop=(ci == NKC - 1),
                    )
                nc.vector.tensor_scalar(
                    qT_sb[:, ss], qp[:, :SBLK], bq_sb[:], 0.0, add, amax
                )

            q_proj(0)
            for sb in range(NSB):
                ss = ds(sb * SBLK, SBLK)
                if sb + 1 < NSB:
                    q_proj(sb + 1)
                # [0:65, 0:512] = head0 num+den, [0:65, 512:1024] = head1
                av = ps_av.tile([128, 1024], f32, name="av")
                for ti in range(NT):
                    tt = ds(ti * 128, 128)
                    sc = ps_a.tile([128, 1024], f32, tag="ps_big", name="sc")
                    for hl in range(2):
                        hh = ds(hl * 64, 64)
                        nc.tensor.matmul(
                            sc[:, ts(hl, SBLK)], kT_sb[hh, tt], qT_sb[hh, ss],
                            start=True, stop=True,
                        )
                    ex = expp.tile([128, 1024], bf16, name="ex")
                    nc.scalar.activation(
                        ex[:], sc[:], Exp, bias=shift_sb[:], scale=0.125
                    )
                    for hl in range(2):
                        nc.tensor.matmul(
                            av[0:65, ts(hl, SBLK)],
                            v_sb_r[:, ti, hl, 0:65],
                            ex[:, ts(hl, SBLK)],
                            start=(ti == 0), stop=(ti == NT - 1),
                        )

                # ---- epilogue: normalize + residual + store ----
                j = sb // 2
                mm = ds((sb % 2) * SBLK, SBLK)
                den64 = epi.tile([65, 1024], f32, name="den64")
                nc.vector.tensor_copy(den64[64:65, :], av[64:65, :])
                den0 = epi.tile([1, 1024], f32, name="den0")
                nc.sync.dma_start(den0[:], den64[64:65, :])
                bcd = epi.tile([64, 1024], f32, name="bcd")
                nc.gpsimd.partition_broadcast(bcd[:], den0[:])
                bcs = epi.tile([64, 1024], f32, name="bcs")
                nc.vector.reciprocal_approx_fast(bcs[:], bcd[:])
                qrt = epi.tile([64, 1024], f32, name="qrt")
                for hl in range(2):
                    nc.sync.dma_start(
                        qrt[:, ts(hl, SBLK)], qres_r[hl, :, j, mm]
                    )
                prod = epi.tile([64, 1024], f32, name="prod")
                nc.vector.tensor_tensor(prod[:], av[0:64, :], bcs[:], mult)
                outt = epi.tile([64, 1024], f32, name="outt")
                nc.vector.tensor_tensor(outt[:], prod[:], qrt[:], add)
                for hl in range(2):
                    nc.sync.dma_start(
                        out_r[hl, :, j, mm], outt[:, ts(hl, SBLK)]
                    )

    nc.compile()
    return nc


def _get_nc():
    if "nc" not in _CACHE:
        _CACHE["nc"] = _build_nc()
    return _CACHE["nc"]


def kernel(queries, keys, values, Wq_w, Wq_b, Wk_w, Wk_b, Wv_w, Wv_b, **kw):
    nc = _get_nc()
    q2 = np.asarray(queries, np.float32).reshape(S, H)
    k2 = np.asarray(keys, np.float32).reshape(S, H)
    v2 = np.asarray(values, np.float32).reshape(S, H)
    xqT = np.ascontiguousarray(q2.T).astype(BF16)
    xkT = np.ascontiguousarray(k2.T).astype(BF16)
    xvT = np.ascontiguousarray(v2.T).astype(BF16)

    in_maps = []
    for c in range(NCORES):
        o = slice(OC * c, OC * (c + 1))
        in_maps.append(
            {
                "xq": xqT,
                "xk": xkT,
                "xv": xvT,
                "wq": np.ascontiguousarray(np.asarray(Wq_w)[o].T).astype(BF16),
                "wk": np.ascontiguousarray(np.asarray(Wk_w)[o].T).astype(BF16),
                "wv": np.ascontiguousarray(np.asarray(Wv_w)[o].T).astype(BF16),
                "bq": np.asarray(Wq_b, np.float32)[o].reshape(OC, 1),
                "bk": np.asarray(Wk_b, np.float32)[o].reshape(OC, 1),
                "bv": np.asarray(Wv_b)[o].astype(BF16).reshape(1, OC),
                "qres": np.ascontiguousarray(q2[512 * c : 512 * (c + 1)]),
            }
        )

    res = run_bass_kernel_spmd(
        nc, in_maps, list(range(NCORES)), **_CACHE.get("run_kwargs", {})
    )
    _CACHE["last_results"] = res
    full = np.concatenate([res.results[c]["out"] for c in range(NCORES)], axis=0)
    return full.reshape(1, S, H)
